# revision 28
# baseline (speedup 1.0000x reference)
"""Self-contained Bass/Trainium2 kernel for the 2-layer LSTM + linear head.

Problem: x [2048, 512, 8] -> 2-layer LSTM (H=50, PyTorch gate order i,f,g,o)
-> last hidden state of layer 2 -> linear [1, 50] -> y [2048, 1].

Strategy: pure data parallel over 8 NeuronCores (256 batch rows each). On
each core the batch is further split into NSB independent sub-batch
pipelines (chains) so the serial T=512 recurrence latency is overlapped.

v8: full bf16 datapath (CPU emulation: rel err ~5e-3 vs the 2e-2 gate).

Gate-per-chunk layout: each of the four matmul chunks holds ONE gate for
BOTH layers - layer 0 on partitions 0:64 (50 used), layer 1 on partitions
64:128.  Both layers share one rhs tile R [128, w] (rows 0:50 h0, 64:114
h1, 114:122 x_t, 122 ones; lhsT columns pick the rows each layer reads),
so each gate needs ONE self-loading matmul per chain per step (K=123) -
LdWeights traffic, not modeled by the cost model but very real on
hardware, is what killed the split x-side variant.  x_t rows are DMA'd
straight from HBM into the rhs tile one step ahead (rp pool rotation
gives 2-step prefetch), so the DMA is off the serial path.

The per-step elementwise phase operates on fully-dense [128, w] tiles:
one stt for m = (sig(2zg)-0.5)*i, one mult for v = f*ct, one add for ct',
one tanh (scale=2; cell state is stored halved - tanh's free input scale)
and ONE h = o*th product covering both layers (junk pad rows 50:64 hit
zero weight columns in the next matmul).  The g gate's weights are
pre-scaled by 2 so only Sigmoid is ever applied to the gate tile.  Layer
2 runs one step behind layer 1 (skew) which makes both layers' matmuls
computable in the same iteration.
"""
import numpy as np
import ml_dtypes
import concourse.bacc as bacc
import concourse.mybir as mybir
from concourse.tile import TileContext
from concourse.bass_utils import run_bass_kernel_spmd

f32 = mybir.dt.float32
bf16 = mybir.dt.bfloat16
AF = mybir.ActivationFunctionType
ALU = mybir.AluOpType

H = 50
D = 8
B = 2048
T = 512
NCORES = 8
BC = B // NCORES   # 256 batch rows per core
NSB = 3
V_ON_POOL = False  # place v = f*ct on GpSimd instead of VectorE
MM_ORDER = "gate"   # "chain": per-chain mm+cell blocks; "gate": gate-major
DMA_SPREAD = False  # spread x DMAs over both HWDGE queues (SP + Act)
RP_BUFS = 3
SP_BUFS = 2
TP_BUFS = 3
SPLIT_SIGMA = False  # sigma over [i,f,g] first, o separately

GATES = ("i", "f", "g", "o")

_NC_CACHE = {}


def _set_nsb(n):
    global NSB, SBS, OFFS
    NSB = n
    w = BC // NSB
    SBS = [w + (1 if i < BC - w * NSB else 0) for i in range(NSB)]
    OFFS = [sum(SBS[:i]) for i in range(NSB)]


_set_nsb(NSB)


def _build_nc(repeat=1):
    nc = bacc.Bacc(None, target_bir_lowering=False)

    xT = nc.dram_tensor("xT", [9, T, BC], bf16, kind="ExternalInput")
    wh = {}
    for q in GATES:
        wh[q] = nc.dram_tensor(f"w{q}", [123, 128], bf16,
                               kind="ExternalInput")
    wfin = nc.dram_tensor("wfin", [128, 1], bf16, kind="ExternalInput")
    y = nc.dram_tensor("y", [1, BC], f32, kind="ExternalOutput")

    with TileContext(nc) as tc:
        with (
            tc.tile_pool(name="wp", bufs=1) as wp,
            tc.tile_pool(name="st", bufs=1) as st,
            tc.tile_pool(name="rp", bufs=RP_BUFS) as rp,
            tc.tile_pool(name="sp", bufs=SP_BUFS) as sp,
            tc.tile_pool(name="tp", bufs=TP_BUFS) as tp,
            tc.tile_pool(name="gp", bufs=2, space="PSUM") as gp,
        ):
            W = {}
            for q, dt in wh.items():
                W[q] = wp.tile([123, 128], bf16, name=f"W{q}")
                nc.sync.dma_start(out=W[q], in_=dt[:, :])
            WF = wp.tile([128, 1], bf16, name="WF")
            nc.sync.dma_start(out=WF, in_=wfin[:, :])

            for _rep in range(repeat):
                _lstm_body(nc, st, rp, sp, tp, gp, xT, W, WF, y)

    nc.compile()
    return nc


def _lstm_body(nc, st, rp, sp, tp, gp, xT, W, WF, y):
    C = [st.tile([128, SBS[sb]], bf16, name=f"C{sb}") for sb in range(NSB)]
    TH = [st.tile([128, SBS[sb]], bf16, name=f"TH{sb}") for sb in range(NSB)]
    for sb in range(NSB):
        nc.vector.memset(C[sb], 0.0)

    def new_r(sb, t, memset):
        r = rp.tile([128, SBS[sb]], bf16, name=f"rt{sb}", tag=f"r_{sb}")
        if memset:
            nc.vector.memset(r, 0.0)
        eng = nc.scalar if (DMA_SPREAD and sb == NSB - 1) else nc.sync
        eng.dma_start(
            out=r[114:123, :],
            in_=xT[0:9, min(t, T - 1), OFFS[sb]:OFFS[sb] + SBS[sb]])
        return r

    rcur = [new_r(sb, 0, True) for sb in range(NSB)]

    def mms(g, sb, rcur):
        w = SBS[sb]
        for qi in range(4):
            nc.tensor.matmul(g[sb][:, qi * w:(qi + 1) * w],
                             W[GATES[qi]][:, :], rcur[sb][0:123, :],
                             start=True, stop=True)

    def cell(g, sb, rnext):
        w = SBS[sb]
        s = sp.tile([128, 4 * w], bf16, name=f"s{sb}", tag=f"s{sb}")
        if SPLIT_SIGMA:
            nc.scalar.activation(out=s[:, 0:3 * w], in_=g[sb][:, 0:3 * w],
                                 func=AF.Sigmoid)
            nc.scalar.activation(out=s[:, 3 * w:4 * w],
                                 in_=g[sb][:, 3 * w:4 * w], func=AF.Sigmoid)
        else:
            nc.scalar.activation(out=s, in_=g[sb][:, :], func=AF.Sigmoid)

        m = tp.tile([128, w], bf16, name=f"m{sb}", tag=f"m{sb}")
        v = tp.tile([128, w], bf16, name=f"v{sb}", tag=f"v{sb}")
        # m = (sig2g - 0.5) * i = i*tanh(zg)/2   (DVE stt)
        nc.vector.scalar_tensor_tensor(out=m, in0=s[:, 2 * w:3 * w],
                                       scalar=0.5, in1=s[:, 0:w],
                                       op0=ALU.subtract, op1=ALU.mult)
        # v = f * ct    (ct = c/2 cell state)
        veng = nc.gpsimd if V_ON_POOL else nc.vector
        veng.tensor_tensor(out=v, in0=s[:, w:2 * w],
                           in1=C[sb], op=ALU.mult)
        # ct' = m + v
        nc.vector.tensor_tensor(out=C[sb], in0=m, in1=v, op=ALU.add)
        # th = tanh(2*ct')
        nc.scalar.activation(out=TH[sb], in_=C[sb], func=AF.Tanh,
                             scale=2.0)
        # h = o * th, both layers at once (junk pad rows are harmless:
        # they hit zero-weight lhsT columns in the next matmul)
        nc.vector.tensor_tensor(out=rnext[sb][0:114, :],
                                in0=s[0:114, 3 * w:4 * w],
                                in1=TH[sb][0:114, :], op=ALU.mult)

    for t in range(T + 1):
        rnext = [new_r(sb, t + 1, t + 1 <= 2) for sb in range(NSB)]
        g = [gp.tile([128, 4 * SBS[sb]], f32, name=f"g{sb}", tag=f"g{sb}")
             for sb in range(NSB)]
        if MM_ORDER == "chain":
            for sb in range(NSB):
                mms(g, sb, rcur)
                cell(g, sb, rnext)
        else:
            for sb in range(NSB):
                mms(g, sb, rcur)
            for sb in range(NSB):
                cell(g, sb, rnext)

        if t == 0:
            # layer 2 ran on junk at t=0 (its real step 0 happens at t=1)
            for sb in range(NSB):
                nc.vector.memset(C[sb][64:128, :], 0.0)
                nc.vector.memset(rnext[sb][64:114, :], 0.0)
        rcur = rnext

    ysb = st.tile([1, BC], f32, name="ysb")
    for sb in range(NSB):
        w = SBS[sb]
        fin = gp.tile([1, w], f32, name=f"fin{sb}", tag=f"g{sb}")
        nc.tensor.matmul(fin[:, :], WF[64:114, :], rcur[sb][64:114, :],
                         start=True, stop=True)
        nc.scalar.copy(out=ysb[:, OFFS[sb]:OFFS[sb] + w], in_=fin[:, :])
    nc.sync.dma_start(out=y[:, :], in_=ysb)


def _prep_weights(Wih0, Whh0, bih0, bhh0, Wih1, Whh1, bih1, bhh1):
    """Per-gate lhsT blobs [123, 128] (bf16), both layers in one tile.

    K-rows match the rhs tile R: 0:50 h0, 64:114 h1, 114:122 x_t, 122
    ones.  L0 output cols 0:50: Whh0^T on h0 rows, Wih0^T on x rows, b0 on
    the ones row.  L1 output cols 64:114: Wih1^T on h0 rows, Whh1^T on h1
    rows, b1 on the ones row.  The g gate is pre-scaled by 2
    (tanh-via-sigmoid trick).
    """
    b0 = (np.asarray(bih0) + np.asarray(bhh0)).astype(np.float32)
    b1 = (np.asarray(bih1) + np.asarray(bhh1)).astype(np.float32)
    Wih0 = np.asarray(Wih0); Whh0 = np.asarray(Whh0)
    Wih1 = np.asarray(Wih1); Whh1 = np.asarray(Whh1)

    out = {}
    for qi, q in enumerate(GATES):
        sc = 2.0 if q == "g" else 1.0
        rows = slice(qi * H, (qi + 1) * H)
        wq = np.zeros((123, 128), np.float32)
        wq[0:50, 0:50] = Whh0[rows, :].T * sc
        wq[0:50, 64:114] = Wih1[rows, :].T * sc
        wq[64:114, 64:114] = Whh1[rows, :].T * sc
        wq[114:122, 0:50] = Wih0[rows, :].T * sc
        wq[122, 0:50] = b0[rows] * sc
        wq[122, 64:114] = b1[rows] * sc
        out[f"w{q}"] = wq.astype(ml_dtypes.bfloat16)
    return out


def _make_in_maps(x, Wih0, Whh0, bih0, bhh0, Wih1, Whh1, bih1, bhh1,
                  Wlin, blin):
    x = np.asarray(x, dtype=np.float32)
    wd = _prep_weights(Wih0, Whh0, bih0, bhh0, Wih1, Whh1, bih1, bhh1)
    wfin = np.zeros((128, 1), np.float32)
    wfin[64:114, 0] = np.asarray(Wlin, dtype=np.float32)[0, :]
    wfin = wfin.astype(ml_dtypes.bfloat16)

    in_maps = []
    for c in range(NCORES):
        xc = x[c * BC:(c + 1) * BC]              # [BC, T, D]
        xt = np.zeros((9, T, BC), dtype=np.float32)
        xt[0:D] = xc.transpose(2, 1, 0)
        xt[D] = 1.0                              # ones row (bias)
        im = {"xT": xt.astype(ml_dtypes.bfloat16), "wfin": wfin}
        im.update(wd)
        in_maps.append(im)
    return in_maps


def kernel(x, Wih0, Whh0, bih0, bhh0, Wih1, Whh1, bih1, bhh1, Wlin, blin):
    in_maps = _make_in_maps(x, Wih0, Whh0, bih0, bhh0, Wih1, Whh1,
                            bih1, bhh1, Wlin, blin)
    if "nc" not in _NC_CACHE:
        _NC_CACHE["nc"] = _build_nc()
    nc = _NC_CACHE["nc"]

    res = run_bass_kernel_spmd(nc, in_maps, core_ids=list(range(NCORES)))
    out = np.empty((B, 1), dtype=np.float32)
    blin_v = np.float32(np.asarray(blin).reshape(-1)[0])
    for c in range(NCORES):
        out[c * BC:(c + 1) * BC, 0] = res.results[c]["y"][0] + blin_v
    return out
